# revision 1
# baseline (speedup 1.0000x reference)
"""BetterCrossCoder (top-k masked autoencoder) Trainium2 Bass kernel.

Computes, for B=2048, D=2048, H=32768, k=32:
    lat = topk_mask(x @ enc + enc_bias, k=32)      # keep top-32 per row
    out = lat @ dec + dec_bias
with enc/dec selected by in_model/out_model.

Strategy (8 NeuronCores, data-parallel over the batch):
  * each core takes 256 batch rows; encoder/decoder weights are replicated.
  * encode: fp32 matmuls ([128,512] PSUM tiles, K accumulated 16x128).
    fp32 is required: the reference selects top-k on exact fp32 scores, and
    bf16/tf32 encodes flip near-threshold selections (measured 2-8% rms
    output error from swapped decoder rows).
  * top-32 per row fully on-chip: each 512-wide score chunk is reduced to
    its top-16 values + in-chunk offsets with the DVE max8/max_index/
    match_replace instructions (chunk scores are discarded immediately -
    the [128, 32768] score matrix never exists in memory).  A second
    max8 cascade over the [128, 1024] candidate array yields the top-32
    values W and candidate positions P; global H-indices are reconstructed
    with an iota chunk-base table + gpsimd indirect_copy + small
    DRAM-bounce DMAs (diagonal extraction).
  * decode: sparse.  For each group of 4 batch rows, dma_gather fetches the
    128 selected decoder rows (float32r) into SBUF; a block-diagonal
    [128, 32] values matrix turns the per-row weighted sums into full-rate
    float32r matmuls accumulated in [32, 512] PSUM tiles (~0.5% of the
    dense decode FLOPs).

Biases are structurally zero for this problem (spec fill=zeros); if a
nonzero bias is ever supplied the kernel falls back to a numpy path.
"""
import sys
sys.path.insert(0, '/opt/trn_rl_repo')
import dataclasses as _dc
import numpy as np

import concourse.bass as bass
import concourse.tile as tile
from concourse import bacc, mybir
from concourse.bass_utils import run_bass_kernel_spmd

F32 = mybir.dt.float32
F32R = mybir.dt.float32r
U16 = mybir.dt.uint16
I16 = mybir.dt.int16
NEG = -1e30

N_CORES = 8
B, D, H, TOPK = 2048, 2048, 32768, 32
B_LOC = B // N_CORES            # 256 rows per core
KCH = D // 128                  # 16 K-chunks
NB = H // 512                   # 64 score chunks
TILES = B_LOC // 128            # 2 batch tiles per core
NDEC = D // 512                 # 4 decode output chunks

_cached = {}


def _build(nc, tc):
    d_xT = nc.dram_tensor("xT", [D, B_LOC], F32, kind="ExternalInput").ap()
    d_enc = nc.dram_tensor("enc", [D, H], F32, kind="ExternalInput").ap()
    d_dec = nc.dram_tensor("dec", [H, D], F32R, kind="ExternalInput").ap()
    d_out = nc.dram_tensor("out", [B_LOC, D], F32, kind="ExternalOutput").ap()

    import contextlib
    ctx = contextlib.ExitStack()
    with ctx:
        const = ctx.enter_context(tc.tile_pool(name="const", bufs=1))
        encp = ctx.enter_context(tc.tile_pool(name="encp", bufs=3))
        hsp = ctx.enter_context(tc.tile_pool(name="hsp", bufs=4))
        candp = ctx.enter_context(tc.tile_pool(name="candp", bufs=1))
        smallp = ctx.enter_context(tc.tile_pool(name="smallp", bufs=2))
        gp = ctx.enter_context(tc.tile_pool(name="gp", bufs=3))
        outp = ctx.enter_context(tc.tile_pool(name="outp", bufs=2))
        psenc = ctx.enter_context(tc.tile_pool(name="psenc", bufs=3, space="PSUM"))
        psdec = ctx.enter_context(tc.tile_pool(name="psdec", bufs=4, space="PSUM"))
        dramp = ctx.enter_context(tc.tile_pool(name="dramp", bufs=2, space="DRAM"))

        xT_sb = const.tile([128, KCH * B_LOC], F32)
        nc.sync.dma_start(xT_sb[:].rearrange("p (k b) -> p k b", k=KCH),
                          d_xT.rearrange("(k p) b -> p k b", p=128))
        base_t = const.tile([128, NB * 16], U16)
        nc.gpsimd.iota(base_t[:], [[512, NB], [0, 16]], base=0, channel_multiplier=0)

        cand_vals = [candp.tile([128, NB * 16], F32, tag=f"cv{m}", name=f"cv{m}")
                     for m in range(TILES)]
        cand_idx = [candp.tile([128, NB * 16], U16, tag=f"ci{m}", name=f"ci{m}")
                    for m in range(TILES)]
        idxw = [candp.tile([128, 8 * 32], I16, tag=f"ixw{m}", name=f"ixw{m}")
                for m in range(TILES)]
        bd = [candp.tile([128, 1024], F32R, tag=f"bd{m}", name=f"bd{m}")
              for m in range(TILES)]

        def encode_tile(m):
            for nb in range(NB):
                et = encp.tile([128, KCH * 512], F32, tag="enc")
                esrc = d_enc[:, 512 * nb:512 * (nb + 1)].rearrange(
                    "(k p) n -> p k n", p=128)
                nc.sync.dma_start(et[:].rearrange("p (k n) -> p k n", k=KCH), esrc)
                pm = psenc.tile([128, 512], F32, tag="pe")
                for k in range(KCH):
                    nc.tensor.matmul(
                        pm[:], xT_sb[:, k * B_LOC + 128 * m: k * B_LOC + 128 * m + 128],
                        et[:, 512 * k:512 * (k + 1)],
                        start=(k == 0), stop=(k == KCH - 1))
                hs = hsp.tile([128, 512], F32, tag="hs")
                nc.scalar.copy(hs[:], pm[:])
                cv8 = cand_vals[m][:, 16 * nb:16 * nb + 8]
                nc.vector.max(cv8, hs[:])
                nc.vector.max_index(cand_idx[m][:, 16 * nb:16 * nb + 8], cv8, hs[:])
                hs2 = hsp.tile([128, 512], F32, tag="hs2")
                nc.vector.match_replace(hs2[:], cv8, hs[:], NEG)
                cv8b = cand_vals[m][:, 16 * nb + 8:16 * nb + 16]
                nc.vector.max(cv8b, hs2[:])
                nc.vector.max_index(cand_idx[m][:, 16 * nb + 8:16 * nb + 16], cv8b, hs2[:])

        def select_tile(m):
            NCAND = NB * 16
            comb = smallp.tile([128, NCAND], U16, tag="comb")
            nc.vector.tensor_tensor(comb[:], base_t[:, :NCAND], cand_idx[m][:],
                                    op=mybir.AluOpType.add)
            W = smallp.tile([128, 32], F32, tag="W")
            P = smallp.tile([128, 32], U16, tag="P")
            scratch = smallp.tile([128, NCAND], F32, tag="cvs")
            bufs = [cand_vals[m], scratch]
            for r in range(4):
                cur = bufs[r % 2]
                nc.vector.max(W[:, 8 * r:8 * r + 8], cur[:])
                nc.vector.max_index(P[:, 8 * r:8 * r + 8], W[:, 8 * r:8 * r + 8], cur[:])
                if r < 3:
                    nc.vector.match_replace(bufs[(r + 1) % 2][:], W[:, 8 * r:8 * r + 8],
                                            cur[:], NEG)
            # PW: P in the per-16-row-group wrapped entry order indirect_copy reads
            p_dram = dramp.tile([128, 32], U16, tag="pd")
            nc.sync.dma_start(p_dram[:], P[:])
            pw = smallp.tile([128, 32], U16, tag="pw")
            pd_flat = p_dram[:].rearrange("p f -> (p f)")
            for q in range(8):
                srcap = pd_flat[512 * q: 512 * (q + 1)].rearrange(
                    "(u a b) -> b u a", u=16, a=2, b=16)
                nc.sync.dma_start(
                    pw[16 * q:16 * (q + 1), :].rearrange("b (u a) -> b u a", a=2),
                    srcap)
            # XG[r, 32u+j] = COMB[r, P[16q+u, j]] for every r in 16-row group q
            xg = smallp.tile([128, 512], U16, tag="xg")
            nc.gpsimd.indirect_copy(xg[:], comb[:], pw[:], True)
            # row r's own indices live on the diagonal: GIDX[r,k] = XG[r, 32(r%16)+k]
            xg_dram = dramp.tile([128, 512], U16, tag="xgd")
            nc.sync.dma_start(xg_dram[:], xg[:])
            gidx_dram = dramp.tile([128, 32], U16, tag="gidxd")
            xgd_flat = xg_dram[:].rearrange("p f -> (p f)")
            diag_src = _dc.replace(xgd_flat, ap=[[8192, 8], [544, 16], [1, 32]])
            nc.sync.dma_start(
                gidx_dram[:].rearrange("(q u) k -> q u k", q=8), diag_src)
            # IDXW[16rep+b, 8g+2j+a] = GIDX[4g+j, 16a+b]  (dma_gather layout)
            gidx_flat = gidx_dram[:].rearrange("p f -> (p f)").bitcast(I16)
            wrap_src = _dc.replace(gidx_flat,
                                   ap=[[1, 16], [128, 32], [32, 4], [16, 2]])
            for rep in range(8):
                nc.sync.dma_start(
                    idxw[m][16 * rep:16 * (rep + 1), :].rearrange(
                        "b (g j a) -> b g j a", g=32, j=4),
                    wrap_src)
            # BD[32j+k, 32g + 4*(g%8) + j] = W[4g+j, k]
            w_dram = dramp.tile([128, 32], F32, tag="wd")
            nc.sync.dma_start(w_dram[:], W[:])
            nc.vector.memset(bd[m][:].bitcast(F32), 0.0)
            w3 = w_dram[:].rearrange("(b s j) k -> j b s k", b=4, s=8, j=4)
            for j in range(4):
                for b in range(4):
                    dst = bd[m][32 * j:32 * (j + 1),
                                256 * b + j: 256 * b + j + 36 * 7 + 1:36]
                    nc.sync.dma_start(dst, w3[j, b].rearrange("s k -> k s").bitcast(F32R))

        def decode_tile(m):
            out_sb = outp.tile([128, D], F32, tag="osb")
            for b32 in range(4):
                pds = [psdec.tile([32, 512], F32, tag="pd", name=f"pd{n}")
                       for n in range(NDEC)]
                for s in range(8):
                    g = 8 * b32 + s
                    gt = gp.tile([128, 1, D], F32R, tag="g")
                    nc.gpsimd.dma_gather(gt[:, :, :], d_dec,
                                         idxw[m][:, 8 * g:8 * (g + 1)],
                                         num_idxs=128, num_idxs_reg=128,
                                         elem_size=D)
                    for n in range(NDEC):
                        nc.tensor.matmul(
                            pds[n][:], bd[m][:, 32 * g:32 * (g + 1)],
                            gt[:, 0, 512 * n:512 * (n + 1)],
                            start=(s == 0), stop=(s == 7))
                for n in range(NDEC):
                    nc.scalar.copy(out_sb[32 * b32:32 * (b32 + 1),
                                          512 * n:512 * (n + 1)], pds[n][:])
            nc.sync.dma_start(d_out[128 * m:128 * (m + 1), :], out_sb[:])

        encode_tile(0)
        select_tile(0)
        encode_tile(1)
        select_tile(1)
        decode_tile(0)
        decode_tile(1)


def _get_module():
    if "nc" not in _cached:
        nc = bacc.Bacc("TRN2", target_bir_lowering=False, debug=False,
                       num_devices=N_CORES)
        with tile.TileContext(nc) as tc:
            _build(nc, tc)
        nc.finalize()
        _cached["nc"] = nc
    return _cached["nc"]


def _numpy_fallback(x, enc, enc_bias, dec, dec_bias):
    h = x.astype(np.float32) @ enc.astype(np.float32) + enc_bias
    idx = np.argpartition(-h, TOPK, axis=1)[:, :TOPK]
    out = np.empty((x.shape[0], dec.shape[1]), np.float32)
    for r in range(x.shape[0]):
        out[r] = h[r, idx[r]] @ dec[idx[r]]
    return out + dec_bias


def kernel(x, enc_a, enc_a_bias, dec_a, dec_a_bias,
           enc_b, enc_b_bias, dec_b, dec_b_bias, in_model, out_model):
    x = np.asarray(x, dtype=np.float32)
    im = int(np.asarray(in_model))
    om = int(np.asarray(out_model))
    enc = np.asarray(enc_a if im == 0 else enc_b, dtype=np.float32)
    enc_bias = np.asarray(enc_a_bias if im == 0 else enc_b_bias, dtype=np.float32)
    dec = np.asarray(dec_a if om == 0 else dec_b, dtype=np.float32)
    dec_bias = np.asarray(dec_a_bias if om == 0 else dec_b_bias, dtype=np.float32)

    if np.any(enc_bias) or np.any(dec_bias):
        return _numpy_fallback(x, enc, enc_bias, dec, dec_bias)

    nc = _get_module()
    enc_c = np.ascontiguousarray(enc)
    dec_c = np.ascontiguousarray(dec)
    in_maps = []
    for c in range(N_CORES):
        xs = x[B_LOC * c:B_LOC * (c + 1)]
        in_maps.append({
            "xT": np.ascontiguousarray(xs.T),
            "enc": enc_c,
            "dec": dec_c,
        })
    res = run_bass_kernel_spmd(nc, in_maps, list(range(N_CORES)))
    return np.concatenate([res.results[c]["out"] for c in range(N_CORES)], axis=0)



# revision 2
# speedup vs baseline: 1.3317x; 1.3317x over previous
"""BetterCrossCoder (top-k masked autoencoder) Trainium2 Bass kernel.

Computes, for B=2048, D=2048, H=32768, k=32:
    lat = topk_mask(x @ enc + enc_bias, k=32)      # keep top-32 per row
    out = lat @ dec + dec_bias
with enc/dec selected by in_model/out_model.

Strategy (8 NeuronCores, data-parallel over the batch):
  * each core takes 256 batch rows; encoder/decoder weights are replicated.
  * encode: exact-fp32 scores from three f32r matmul volumes at ~1 PE
    cycle/row each (vs 4 for native fp32).  f32r truncates operands to 12
    significand bits but accumulates exactly, so with the splits
    xh = f32r(x), xl = x - xh (12-bit residue), eh = f32r(e), el = e - eh,
      score = xh@eh + xh@el + xl@eh            (xl@el ~ 1e-9, dropped)
    every operand is 12-bit-clean and passes through the f32r input
    rounding unchanged -> bit-level agreement with the fp32 reference to
    ~1.8e-7 (the fp32 accumulation-order noise floor).  Exactness matters:
    one flipped top-k selection swaps a whole decoder row (~0.14 rel err).
  * enc is streamed block-major (each [2048,512] block loaded once, used
    by both 128-row tiles; 256MB/core), in quarter-block rings with eh/el
    produced on scalar/vector engines under the tensor shadow.
  * top-32 per row fully on-chip: per 512-chunk top-16 via DVE
    max8/max_index/match_replace reading PSUM directly, then a 4-round
    cascade over the [128, 1024] candidates; global H-indices are
    reconstructed with an iota chunk-base table + gpsimd indirect_copy +
    small DRAM-bounce DMAs (diagonal extraction), with the two row-tiles'
    selects step-interleaved so their DMA latency chains overlap.
  * decode: sparse.  For each group of 4 batch rows, dma_gather fetches the
    128 selected decoder rows (float32r) into SBUF; a block-diagonal
    [128, 32] values matrix turns the per-row weighted sums into full-rate
    float32r matmuls accumulated in [32, 512] PSUM tiles.

Biases are structurally zero for this problem (spec fill=zeros); if a
nonzero bias is ever supplied the kernel falls back to a numpy path.
"""
import sys
sys.path.insert(0, '/opt/trn_rl_repo')
import dataclasses as _dc
import numpy as np

import concourse.bass as bass
import concourse.tile as tile
from concourse import bacc, mybir
from concourse.bass_utils import run_bass_kernel_spmd

F32 = mybir.dt.float32
F32R = mybir.dt.float32r
U16 = mybir.dt.uint16
I16 = mybir.dt.int16
NEG = -1e30

N_CORES = 8
B, D, H, TOPK = 2048, 2048, 32768, 32
B_LOC = B // N_CORES            # 256 rows per core
KCH = D // 128                  # 16 K-chunks
NB = H // 512                   # 64 score chunks
TILES = B_LOC // 128            # 2 batch tiles per core
NDEC = D // 512                 # 4 decode output chunks
QK = 4                          # k-chunks per streamed quarter-block
NQ = KCH // QK

_cached = {}


def _build(nc, tc):
    d_xT = nc.dram_tensor("xT", [D, B_LOC], F32, kind="ExternalInput").ap()
    d_enc = nc.dram_tensor("enc", [D, H], F32, kind="ExternalInput").ap()
    d_dec = nc.dram_tensor("dec", [H, D], F32R, kind="ExternalInput").ap()
    d_out = nc.dram_tensor("out", [B_LOC, D], F32, kind="ExternalOutput").ap()

    import contextlib
    ctx = contextlib.ExitStack()
    with ctx:
        const = ctx.enter_context(tc.tile_pool(name="const", bufs=1))
        encp = ctx.enter_context(tc.tile_pool(name="encp", bufs=2))
        ehp = ctx.enter_context(tc.tile_pool(name="ehp", bufs=3))
        elp = ctx.enter_context(tc.tile_pool(name="elp", bufs=3))
        hsp = ctx.enter_context(tc.tile_pool(name="hsp", bufs=2))
        candp = ctx.enter_context(tc.tile_pool(name="candp", bufs=1))
        smallp = ctx.enter_context(tc.tile_pool(name="smallp", bufs=1))
        gp = ctx.enter_context(tc.tile_pool(name="gp", bufs=3))
        outp = ctx.enter_context(tc.tile_pool(name="outp", bufs=1))
        psenc = ctx.enter_context(tc.tile_pool(name="psenc", bufs=3, space="PSUM"))
        psdec = ctx.enter_context(tc.tile_pool(name="psdec", bufs=4, space="PSUM"))
        dramp = ctx.enter_context(tc.tile_pool(name="dramp", bufs=2, space="DRAM"))

        xT_sb = const.tile([128, KCH * B_LOC], F32)
        nc.sync.dma_start(xT_sb[:].rearrange("p (k b) -> p k b", k=KCH),
                          d_xT.rearrange("(k p) b -> p k b", p=128))
        base_t = const.tile([128, NB * 16], U16)
        nc.gpsimd.iota(base_t[:], [[512, NB], [0, 16]], base=0, channel_multiplier=0)

        cand_vals = [candp.tile([128, NB * 16], F32, tag=f"cv{m}", name=f"cv{m}")
                     for m in range(TILES)]
        cand_idx = [candp.tile([128, NB * 16], U16, tag=f"ci{m}", name=f"ci{m}")
                    for m in range(TILES)]
        idxw = [candp.tile([128, 8 * 32], I16, tag=f"ixw{m}", name=f"ixw{m}")
                for m in range(TILES)]
        bd = [candp.tile([128, 1024], F32R, tag=f"bd{m}", name=f"bd{m}")
              for m in range(TILES)]

        def topk_block_psum(m, nb, pm):
            cv8 = cand_vals[m][:, 16 * nb:16 * nb + 8]
            nc.vector.max(cv8, pm[:])
            nc.vector.max_index(cand_idx[m][:, 16 * nb:16 * nb + 8], cv8, pm[:])
            hs2 = hsp.tile([128, 512], F32, tag="hs2", name="hs2")
            nc.vector.match_replace(hs2[:], cv8, pm[:], NEG)
            cv8b = cand_vals[m][:, 16 * nb + 8:16 * nb + 16]
            nc.vector.max(cv8b, hs2[:])
            nc.vector.max_index(cand_idx[m][:, 16 * nb + 8:16 * nb + 16], cv8b, hs2[:])

        def encode_blocks_exact():
            xh = const.tile([128, KCH * B_LOC], F32R, name="xh")
            nc.scalar.copy(xh[:], xT_sb[:])
            xl = const.tile([128, KCH * B_LOC], F32R, name="xl")
            nc.vector.tensor_tensor(xl[:], xT_sb[:], xh[:],
                                    op=mybir.AluOpType.subtract)
            for nb in range(NB):
                pms = [psenc.tile([128, 512], F32, tag="pe", name=f"pm{m}")
                       for m in range(TILES)]
                for q in range(NQ):
                    etq = encp.tile([128, QK * 512], F32, tag="enc", name="etq")
                    esrc = d_enc[128 * QK * q:128 * QK * (q + 1),
                                 512 * nb:512 * (nb + 1)].rearrange(
                        "(k p) n -> p k n", p=128)
                    nc.sync.dma_start(
                        etq[:].rearrange("p (k n) -> p k n", k=QK), esrc)
                    ehq = ehp.tile([128, QK * 512], F32R, tag="eh", name="ehq")
                    nc.scalar.copy(ehq[:], etq[:])
                    elq = elp.tile([128, QK * 512], F32R, tag="el", name="elq")
                    nc.vector.tensor_tensor(elq[:], etq[:], ehq[:],
                                            op=mybir.AluOpType.subtract)
                    for m in range(TILES):
                        xo = 128 * m
                        for kq in range(QK):
                            k = QK * q + kq
                            xs = slice(k * B_LOC + xo, k * B_LOC + xo + 128)
                            first = (q == 0 and kq == 0)
                            last = (q == NQ - 1 and kq == QK - 1)
                            nc.tensor.matmul(
                                pms[m][:], xh[:, xs],
                                ehq[:, 512 * kq:512 * (kq + 1)],
                                start=first, stop=False)
                            nc.tensor.matmul(
                                pms[m][:], xh[:, xs],
                                elq[:, 512 * kq:512 * (kq + 1)],
                                start=False, stop=False)
                            nc.tensor.matmul(
                                pms[m][:], xl[:, xs],
                                ehq[:, 512 * kq:512 * (kq + 1)],
                                start=False, stop=last)
                for m in range(TILES):
                    topk_block_psum(m, nb, pms[m])

        def select_tiles(ms):
            NCAND = NB * 16
            st = {}
            for m in ms:
                comb = smallp.tile([128, NCAND], U16, tag=f"comb{m}", name=f"comb{m}")
                nc.vector.tensor_tensor(comb[:], base_t[:, :NCAND], cand_idx[m][:],
                                        op=mybir.AluOpType.add)
                st[m] = {"comb": comb}
            for m in ms:
                W = smallp.tile([128, 32], F32, tag=f"W{m}", name=f"W{m}")
                P = smallp.tile([128, 32], U16, tag=f"P{m}", name=f"P{m}")
                scratch = smallp.tile([128, NCAND], F32, tag=f"cvs{m}", name=f"cvs{m}")
                bufs = [cand_vals[m], scratch]
                for r in range(4):
                    cur = bufs[r % 2]
                    nc.vector.max(W[:, 8 * r:8 * r + 8], cur[:])
                    nc.vector.max_index(P[:, 8 * r:8 * r + 8], W[:, 8 * r:8 * r + 8],
                                        cur[:])
                    if r < 3:
                        nc.vector.match_replace(bufs[(r + 1) % 2][:],
                                                W[:, 8 * r:8 * r + 8], cur[:], NEG)
                st[m]["W"], st[m]["P"] = W, P
            for m in ms:
                p_dram = dramp.tile([128, 32], U16, tag=f"pd{m}", name=f"pdr{m}")
                nc.sync.dma_start(p_dram[:], st[m]["P"][:])
                st[m]["p_dram"] = p_dram
                w_dram = dramp.tile([128, 32], F32, tag=f"wd{m}", name=f"wdr{m}")
                nc.sync.dma_start(w_dram[:], st[m]["W"][:])
                st[m]["w_dram"] = w_dram
                nc.vector.memset(bd[m][:].bitcast(F32), 0.0)
            for m in ms:
                # PW: P in the per-16-row-group wrapped entry order
                pw = smallp.tile([128, 32], U16, tag=f"pw{m}", name=f"pw{m}")
                pd_flat = st[m]["p_dram"][:].rearrange("p f -> (p f)")
                for q in range(8):
                    srcap = pd_flat[512 * q: 512 * (q + 1)].rearrange(
                        "(u a b) -> b u a", u=16, a=2, b=16)
                    nc.sync.dma_start(
                        pw[16 * q:16 * (q + 1), :].rearrange("b (u a) -> b u a", a=2),
                        srcap)
                st[m]["pw"] = pw
                # BD[32j+k, 32g + 4*(g%8) + j] = W[4g+j, k]
                w3 = st[m]["w_dram"][:].rearrange("(b s j) k -> j b s k", b=4, s=8, j=4)
                for j in range(4):
                    for b in range(4):
                        dst = bd[m][32 * j:32 * (j + 1),
                                    256 * b + j: 256 * b + j + 36 * 7 + 1:36]
                        nc.sync.dma_start(dst,
                                          w3[j, b].rearrange("s k -> k s").bitcast(F32R))
            for m in ms:
                # XG[r, 32u+j] = COMB[r, P[16q+u, j]] for every r in 16-row group q
                xg = smallp.tile([128, 512], U16, tag=f"xg{m}", name=f"xg{m}")
                nc.gpsimd.indirect_copy(xg[:], st[m]["comb"][:], st[m]["pw"][:], True)
                xg_dram = dramp.tile([128, 512], U16, tag=f"xgd{m}", name=f"xgd{m}")
                nc.sync.dma_start(xg_dram[:], xg[:])
                st[m]["xg_dram"] = xg_dram
            for m in ms:
                # row r's own indices live on the diagonal
                gidx_dram = dramp.tile([128, 32], U16, tag=f"gidxd{m}",
                                       name=f"gidxd{m}")
                xgd_flat = st[m]["xg_dram"][:].rearrange("p f -> (p f)")
                diag_src = _dc.replace(xgd_flat, ap=[[8192, 8], [544, 16], [1, 32]])
                nc.sync.dma_start(
                    gidx_dram[:].rearrange("(q u) k -> q u k", q=8), diag_src)
                st[m]["gidx_dram"] = gidx_dram
            for m in ms:
                # IDXW[16rep+b, 8g+2j+a] = GIDX[4g+j, 16a+b]  (dma_gather layout)
                gidx_flat = st[m]["gidx_dram"][:].rearrange("p f -> (p f)").bitcast(I16)
                wrap_src = _dc.replace(gidx_flat,
                                       ap=[[1, 16], [128, 32], [32, 4], [16, 2]])
                for rep in range(8):
                    nc.sync.dma_start(
                        idxw[m][16 * rep:16 * (rep + 1), :].rearrange(
                            "b (g j a) -> b g j a", g=32, j=4),
                        wrap_src)

        def decode_tile(m):
            out_sb = outp.tile([128, D], F32, tag="osb")
            for b32 in range(4):
                pds = [psdec.tile([32, 512], F32, tag="pd", name=f"pd{n}")
                       for n in range(NDEC)]
                for s in range(8):
                    g = 8 * b32 + s
                    gt = gp.tile([128, 1, D], F32R, tag="g")
                    nc.gpsimd.dma_gather(gt[:, :, :], d_dec,
                                         idxw[m][:, 8 * g:8 * (g + 1)],
                                         num_idxs=128, num_idxs_reg=128,
                                         elem_size=D)
                    for n in range(NDEC):
                        nc.tensor.matmul(
                            pds[n][:], bd[m][:, 32 * g:32 * (g + 1)],
                            gt[:, 0, 512 * n:512 * (n + 1)],
                            start=(s == 0), stop=(s == 7))
                for n in range(NDEC):
                    nc.scalar.copy(out_sb[32 * b32:32 * (b32 + 1),
                                          512 * n:512 * (n + 1)], pds[n][:])
            nc.sync.dma_start(d_out[128 * m:128 * (m + 1), :], out_sb[:])

        encode_blocks_exact()
        select_tiles([0, 1])
        decode_tile(0)
        decode_tile(1)


def _get_module():
    if "nc" not in _cached:
        nc = bacc.Bacc("TRN2", target_bir_lowering=False, debug=False,
                       num_devices=N_CORES)
        with tile.TileContext(nc) as tc:
            _build(nc, tc)
        nc.finalize()
        _cached["nc"] = nc
    return _cached["nc"]


def _numpy_fallback(x, enc, enc_bias, dec, dec_bias):
    h = x.astype(np.float32) @ enc.astype(np.float32) + enc_bias
    idx = np.argpartition(-h, TOPK, axis=1)[:, :TOPK]
    out = np.empty((x.shape[0], dec.shape[1]), np.float32)
    for r in range(x.shape[0]):
        out[r] = h[r, idx[r]] @ dec[idx[r]]
    return out + dec_bias


def kernel(x, enc_a, enc_a_bias, dec_a, dec_a_bias,
           enc_b, enc_b_bias, dec_b, dec_b_bias, in_model, out_model):
    x = np.asarray(x, dtype=np.float32)
    im = int(np.asarray(in_model))
    om = int(np.asarray(out_model))
    enc = np.asarray(enc_a if im == 0 else enc_b, dtype=np.float32)
    enc_bias = np.asarray(enc_a_bias if im == 0 else enc_b_bias, dtype=np.float32)
    dec = np.asarray(dec_a if om == 0 else dec_b, dtype=np.float32)
    dec_bias = np.asarray(dec_a_bias if om == 0 else dec_b_bias, dtype=np.float32)

    if np.any(enc_bias) or np.any(dec_bias):
        return _numpy_fallback(x, enc, enc_bias, dec, dec_bias)

    nc = _get_module()
    enc_c = np.ascontiguousarray(enc)
    dec_c = np.ascontiguousarray(dec)
    in_maps = []
    for c in range(N_CORES):
        xs = x[B_LOC * c:B_LOC * (c + 1)]
        in_maps.append({
            "xT": np.ascontiguousarray(xs.T),
            "enc": enc_c,
            "dec": dec_c,
        })
    res = run_bass_kernel_spmd(nc, in_maps, list(range(N_CORES)))
    return np.concatenate([res.results[c]["out"] for c in range(N_CORES)], axis=0)
